# revision 1
# baseline (speedup 1.0000x reference)
"""Trainium2 Bass kernel for nn_MemoryEfficientBSpline (linear B-spline / KAN layer).

Math: out[b,o,p] = sum_i sum_g Wt[b,i,p,g] * coef[b,o,i,g] where Wt is the
two-hot linear-interpolation weight of x[b,i,p] over a 6-knot grid on [-1,1].

Reformulation (hinge basis): with xc = clip(x,-1,1) and nx = 2.5*(xc+1) in [0,5],
the piecewise-linear spline f(nx) = sum_k beta_k * relu(nx - k) + c0 becomes

  out[b,o,p] = alpha[b,o] + sum_i W0[b,o,i]*xc[b,i,p]
             + sum_{k=1..4} sum_i Wk[b,o,i]*relu(xc[b,i,p] + 1 - 0.4k)

i.e. 5 dense [64x64] matmuls over cheap elementwise "hinge planes" of x.
The one-hot construction disappears entirely; coefficients are transformed
host-side (tiny: [8,64,64,6]).

Sharding: data-parallel over batch B=8, one batch per NeuronCore. Per core the
64x36864 pixel plane is folded to 128 partitions (two pixel-halves stacked), and
the 64x64 weights are block-diagonal-duplicated to 128x128 so one full-array
matmul handles both halves.

Dtype: planes/weights are float32r (fp32 bits, reduced-precision PE multiply,
1 cycle/row) -> ~1e-4 rel err, 4x faster than strict fp32 matmul.
"""
import numpy as np
from contextlib import ExitStack

import concourse.bass as bass
import concourse.tile as tile
from concourse import bacc, mybir
from concourse.bass_utils import run_bass_kernel_spmd

# Problem shapes (hardcoded per contract)
B, IN_DIM, H, W = 8, 64, 192, 192
OUT_DIM = 64
G = 6
P_TOT = H * W          # 36864 pixels
HALF = P_TOT // 2      # 18432 (folded columns)
NPART = 128
SLAB = 2048            # columns processed per pipeline iteration
N_SLABS = HALF // SLAB # 9
CHUNK = 512            # matmul moving-operand size (= one PSUM bank of fp32)
N_CHUNKS = SLAB // CHUNK
NK = 5                 # hinge planes: xc, relu(xc+0.6), relu(xc+0.2), relu(xc-0.2), relu(xc-0.6)

_f32 = mybir.dt.float32
_f32r = mybir.dt.float32r
_Alu = mybir.AluOpType
_Act = mybir.ActivationFunctionType

_cached = None  # compiled Bass module, built once per process


def _build_module(n_reps=1):
    """n_reps>1 wraps the whole body in a hardware loop — used only for
    slope-based HW timing (dispatch noise >> exec time in this env)."""
    nc = bacc.Bacc("TRN2", target_bir_lowering=False, debug=False,
                   enable_asserts=False, num_devices=8)

    x_t = nc.dram_tensor("x", (NPART, HALF), _f32, kind="ExternalInput")
    w_t = nc.dram_tensor("wts", (NK, NPART, NPART), _f32r, kind="ExternalInput")
    b_t = nc.dram_tensor("bias", (NPART, 1), _f32, kind="ExternalInput")
    y_t = nc.dram_tensor("y", (NPART, HALF), _f32, kind="ExternalOutput")

    with tile.TileContext(nc) as tc, ExitStack() as ctx:
        cpool = ctx.enter_context(tc.tile_pool(name="const", bufs=1))
        xpool = ctx.enter_context(tc.tile_pool(name="xin", bufs=4))
        ppool = ctx.enter_context(tc.tile_pool(name="planes", bufs=3))
        opool = ctx.enter_context(tc.tile_pool(name="oslab", bufs=4))
        psum = ctx.enter_context(tc.tile_pool(name="acc", bufs=2, space="PSUM"))

        # Constants: weights [128, 5*128] (block-diag per plane), biases
        wts = cpool.tile([NPART, NK * NPART], _f32r)
        for k in range(NK):
            nc.sync.dma_start(wts[:, k*NPART:(k+1)*NPART], w_t[k])
        bias = cpool.tile([NPART, 1], _f32)
        nc.sync.dma_start(bias[:], b_t[:])
        c_p4 = cpool.tile([NPART, 1], _f32)
        nc.vector.memset(c_p4[:], -0.6)  # ACT-computed plane: relu(x - 0.6) bias

        def body():
            # Software-pipelined: evict of slab s-1 is emitted AFTER the ACT
            # plane of slab s, so the strict-FIFO ACT queue never makes the
            # next slab's plane wait behind the previous slab's psum evict.
            pend = None  # (psum_tile, col0) awaiting evict
            for s in range(N_SLABS + 1):
                if s < N_SLABS:
                    col0 = s * SLAB
                    xt = xpool.tile([NPART, SLAB], _f32)
                    nc.sync.dma_start(xt[:], x_t[:, col0:col0 + SLAB])

                    # Hinge planes (float32r out). relu(xc+c) == clip(x,-c,1)+c,
                    # with the +c shift folded into the output bias host-side —
                    # so DVE planes read raw x directly (no xc dependency chain).
                    # Slab 0 is emitted in 512-col quarters so the first matmul
                    # chunk's inputs are ready ~6us sooner (shorter pipeline fill).
                    pieces = 4 if s == 0 else 1
                    pw = SLAB // pieces
                    planes = []
                    xc = ppool.tile([NPART, SLAB], _f32r, tag="xc")
                    planes.append(xc)
                    for k, cst in ((1, 0.6), (2, 0.2), (3, -0.2)):
                        planes.append(ppool.tile([NPART, SLAB], _f32r,
                                                 tag=f"p{k}", name=f"p{k}"))
                    p4 = ppool.tile([NPART, SLAB], _f32r, tag="p4")
                    for q in range(pieces):
                        sl = slice(q * pw, (q + 1) * pw)
                        nc.vector.tensor_scalar(xc[:, sl], xt[:, sl], 1.0, -1.0,
                                                _Alu.min, _Alu.max)
                        for j, cst in ((1, 0.6), (2, 0.2), (3, -0.2)):
                            nc.vector.tensor_scalar(planes[j][:, sl], xt[:, sl],
                                                    1.0, -cst, _Alu.min, _Alu.max)
                        nc.scalar.activation(p4[:, sl], xc[:, sl], _Act.Relu,
                                             bias=c_p4[:], scale=1.0)
                    planes.append(p4)

                if pend is not None:
                    # Evict + bias in one ACT pass over 4 psum banks, DMA out
                    acc_p, pcol0 = pend
                    ot = opool.tile([NPART, SLAB], _f32)
                    nc.scalar.activation(ot[:], acc_p[:], _Act.Identity, bias=bias[:], scale=1.0)
                    nc.sync.dma_start(y_t[:, pcol0:pcol0 + SLAB], ot[:])
                    pend = None

                if s < N_SLABS:
                    # Matmuls: 5 planes x 4 chunks accumulate into 4-bank psum
                    acc = psum.tile([NPART, SLAB], _f32)
                    for k in range(NK):
                        wk = wts[:, k*NPART:(k+1)*NPART]
                        for c in range(N_CHUNKS):
                            nc.tensor.matmul(acc[:, c*CHUNK:(c+1)*CHUNK], wk,
                                             planes[k][:, c*CHUNK:(c+1)*CHUNK],
                                             start=(k == 0), stop=(k == NK - 1))
                    pend = (acc, col0)

        for _ in range(n_reps):
            body()

    nc.compile()
    return nc


def _get_module():
    global _cached
    if _cached is None:
        _cached = _build_module()
    return _cached


def _prep_inputs(x, coef):
    """Host-side shard + coefficient transform. Returns in_maps for 8 cores."""
    x = np.ascontiguousarray(x, dtype=np.float32)
    c = np.asarray(coef, dtype=np.float64)            # [B, o, i, 6]
    d = np.diff(c, axis=-1)                           # [B, o, i, 5]
    beta = np.concatenate([d[..., :1], np.diff(d, axis=-1)], axis=-1)
    Wk = (2.5 * beta).astype(np.float32)              # [B, o, i, 5]
    Wk64 = Wk.astype(np.float64)
    # Device planes k=1..3 are clip(x,-c_k,1) = relu(xc+c_k) - c_k: fold the
    # +c_k shift into the bias (c_k = 1 - 0.4k).
    alpha = (c[..., 0].sum(axis=2) + Wk64[..., 0].sum(axis=2)
             + sum((1.0 - 0.4 * k) * Wk64[..., k].sum(axis=2) for k in (1, 2, 3))
             ).astype(np.float32)                     # [B, o]

    in_maps = []
    eye2 = np.eye(2, dtype=np.float32)
    for b in range(B):
        xb = x[b].reshape(IN_DIM, P_TOT)
        x_f = np.concatenate([xb[:, :HALF], xb[:, HALF:]], axis=0)  # [128, HALF]
        # lhsT[k][i, o] = Wk[b, o, i, k], block-diag duplicated to 128x128
        lhsT = np.einsum('oik->kio', Wk[b])           # [5, i, o]
        wts = np.kron(eye2, lhsT).astype(np.float32)  # [5, 128, 128]
        bias = np.tile(alpha[b], 2).reshape(NPART, 1).astype(np.float32)
        in_maps.append({
            "x": np.ascontiguousarray(x_f),
            "wts": np.ascontiguousarray(wts),
            "bias": bias,
        })
    return in_maps


def _assemble(results):
    out = np.empty((B, OUT_DIM, H, W), dtype=np.float32)
    for b in range(B):
        y_f = results[b]["y"]                          # [128, HALF]
        out[b] = np.concatenate([y_f[:OUT_DIM], y_f[OUT_DIM:]], axis=1).reshape(OUT_DIM, H, W)
    return out


def run(x, coef, **spmd_kwargs):
    """Run on 8 NeuronCores; returns (output, BassKernelResults)."""
    nc = _get_module()
    in_maps = _prep_inputs(x, coef)
    res = run_bass_kernel_spmd(nc, in_maps, core_ids=list(range(8)), **spmd_kwargs)
    return _assemble(res.results), res


def kernel(x, coef):
    out, _ = run(x, coef)
    return out



# revision 3
# speedup vs baseline: 113.3849x; 113.3849x over previous
"""Trainium2 Bass kernel for nn_MemoryEfficientBSpline (linear B-spline / KAN layer).

Math: out[b,o,p] = sum_i sum_g Wt[b,i,p,g] * coef[b,o,i,g] where Wt is the
two-hot linear-interpolation weight of x[b,i,p] over a 6-knot grid on [-1,1].

Reformulation (hinge basis): with xc = clip(x,-1,1) and nx = 2.5*(xc+1) in [0,5],
the piecewise-linear spline f(nx) = sum_k beta_k * relu(nx - k) + c0 becomes

  out[b,o,p] = alpha[b,o] + sum_i W0[b,o,i]*xc[b,i,p]
             + sum_{k=1..4} sum_i Wk[b,o,i]*relu(xc[b,i,p] + 1 - 0.4k)

i.e. 5 dense [64x64] matmuls over cheap elementwise "hinge planes" of x.
Every device plane is clip(x, lo_k, 1) with the constant shift folded into the
output bias host-side, so all 5 planes are a single DVE tensor_scalar each.

Sharding: data-parallel over batch B=8, one batch per NeuronCore. Per core the
64x36864 pixel plane is folded to 128 partitions (two pixel-halves stacked), and
the 64x64 weights are block-diagonal-duplicated to 128x128 so one full-array
matmul handles both halves.

Dtype: fp16 end-to-end on device — x, hinge planes, weights, and y are all
fp16 (PSUM accumulates fp32; bias add + downcast happen on the ACT evict).
This halves HBM traffic vs fp32 (the old bottleneck) and lets the DVE run its
4x 2-byte perf mode, leaving the PE matmuls (1 cycle/column, same rate as
fp32r) as the bottleneck. fp16's 10 mantissa bits keep rel err ~1e-3.
"""
import numpy as np
from contextlib import ExitStack

import concourse.bass as bass
import concourse.tile as tile
from concourse import bacc, mybir
from concourse.bass_utils import run_bass_kernel_spmd

# Problem shapes (hardcoded per contract)
B, IN_DIM, H, W = 8, 64, 192, 192
OUT_DIM = 64
G = 6
P_TOT = H * W          # 36864 pixels
HALF = P_TOT // 2      # 18432 (folded columns)
NPART = 128
SLAB = 2048            # columns processed per pipeline iteration
N_SLABS = HALF // SLAB # 9
CHUNK = 512            # matmul moving-operand size (= one PSUM bank of fp32)
N_CHUNKS = SLAB // CHUNK
NK = 5                 # planes: clip(x, lo_k, 1) for lo = -1, -.6, -.2, .2, .6
LOS = (-1.0, -0.6, -0.2, 0.2, 0.6)

_f32 = mybir.dt.float32
_f16 = mybir.dt.float16
_Alu = mybir.AluOpType
_Act = mybir.ActivationFunctionType

_cached = None  # compiled Bass module, built once per process


def _build_module(n_reps=1):
    """n_reps>1 wraps the whole body in a hardware loop — used only for
    slope-based HW timing (dispatch noise >> exec time in this env)."""
    nc = bacc.Bacc("TRN2", target_bir_lowering=False, debug=False,
                   enable_asserts=False, num_devices=8)

    x_t = nc.dram_tensor("x", (NPART, HALF), _f16, kind="ExternalInput")
    w_t = nc.dram_tensor("wts", (NK, NPART, NPART), _f16, kind="ExternalInput")
    b_t = nc.dram_tensor("bias", (NPART, 1), _f32, kind="ExternalInput")
    y_t = nc.dram_tensor("y", (NPART, HALF), _f16, kind="ExternalOutput")

    with tile.TileContext(nc) as tc, ExitStack() as ctx:
        cpool = ctx.enter_context(tc.tile_pool(name="const", bufs=1))
        xpool = ctx.enter_context(tc.tile_pool(name="xin", bufs=3))
        ppool = ctx.enter_context(tc.tile_pool(name="planes", bufs=2))
        opool = ctx.enter_context(tc.tile_pool(name="oslab", bufs=3))
        psum = ctx.enter_context(tc.tile_pool(name="acc", bufs=2, space="PSUM"))

        # Constants: weights [128, 5*128] (block-diag per plane), bias
        wts = cpool.tile([NPART, NK * NPART], _f16)
        for k in range(NK):
            nc.sync.dma_start(wts[:, k*NPART:(k+1)*NPART], w_t[k])
        bias = cpool.tile([NPART, 1], _f32)
        nc.sync.dma_start(bias[:], b_t[:])

        def body():
            for s in range(N_SLABS):
                col0 = s * SLAB
                # Slab 0 is loaded and plane-computed in 512-col quarters so
                # the first matmul's inputs are ready sooner (pipeline fill).
                pieces = 4 if s == 0 else 1
                pw = SLAB // pieces
                xt = xpool.tile([NPART, SLAB], _f16)
                planes = [ppool.tile([NPART, SLAB], _f16, tag=f"p{k}",
                                     name=f"p{k}")
                          for k in range(NK)]
                for q in range(pieces):
                    sl = slice(q * pw, (q + 1) * pw)
                    nc.sync.dma_start(xt[:, sl], x_t[:, col0 + q*pw:col0 + (q+1)*pw])
                    for k in range(NK):
                        nc.vector.tensor_scalar(planes[k][:, sl], xt[:, sl],
                                                1.0, LOS[k], _Alu.min, _Alu.max)

                # Matmuls: 5 planes x 4 chunks accumulate into 4-bank psum
                acc = psum.tile([NPART, SLAB], _f32)
                for k in range(NK):
                    wk = wts[:, k*NPART:(k+1)*NPART]
                    for c in range(N_CHUNKS):
                        nc.tensor.matmul(acc[:, c*CHUNK:(c+1)*CHUNK], wk,
                                         planes[k][:, c*CHUNK:(c+1)*CHUNK],
                                         start=(k == 0), stop=(k == NK - 1))

                # Evict + bias in one ACT pass over 4 psum banks, DMA out
                ot = opool.tile([NPART, SLAB], _f16)
                nc.scalar.activation(ot[:], acc[:], _Act.Identity,
                                     bias=bias[:], scale=1.0)
                nc.sync.dma_start(y_t[:, col0:col0 + SLAB], ot[:])

        for _ in range(n_reps):
            body()

    nc.compile()
    return nc


def _get_module():
    global _cached
    if _cached is None:
        _cached = _build_module()
    return _cached


def _prep_inputs(x, coef):
    """Host-side shard + coefficient transform. Returns in_maps for 8 cores."""
    x = np.asarray(x, dtype=np.float32)
    c = np.asarray(coef, dtype=np.float64)            # [B, o, i, 6]
    d = np.diff(c, axis=-1)                           # [B, o, i, 5]
    beta = np.concatenate([d[..., :1], np.diff(d, axis=-1)], axis=-1)
    Wk = (2.5 * beta).astype(np.float16)              # [B, o, i, 5]
    Wk64 = Wk.astype(np.float64)
    # Device planes are clip(x, lo_k, 1) = relu(xc + c_k) - c_k with
    # c_k = 1 - 0.4k = -lo_k: fold the +c_k shift into the bias.
    alpha = (c[..., 0].sum(axis=2) + Wk64[..., 0].sum(axis=2)
             + sum((1.0 - 0.4 * k) * Wk64[..., k].sum(axis=2) for k in (1, 2, 3, 4))
             ).astype(np.float32)                     # [B, o]

    in_maps = []
    eye2 = np.eye(2, dtype=np.float16)
    for b in range(B):
        xb = x[b].reshape(IN_DIM, P_TOT)
        x_f = np.concatenate([xb[:, :HALF], xb[:, HALF:]], axis=0)  # [128, HALF]
        # lhsT[k][i, o] = Wk[b, o, i, k], block-diag duplicated to 128x128
        lhsT = np.einsum('oik->kio', Wk[b])           # [5, i, o]
        wts = np.kron(eye2, lhsT).astype(np.float16)  # [5, 128, 128]
        bias = np.tile(alpha[b], 2).reshape(NPART, 1).astype(np.float32)
        in_maps.append({
            "x": np.ascontiguousarray(x_f.astype(np.float16)),
            "wts": np.ascontiguousarray(wts),
            "bias": bias,
        })
    return in_maps


def _assemble(results):
    out = np.empty((B, OUT_DIM, H, W), dtype=np.float32)
    for b in range(B):
        y_f = results[b]["y"].astype(np.float32)       # [128, HALF]
        out[b] = np.concatenate([y_f[:OUT_DIM], y_f[OUT_DIM:]], axis=1).reshape(OUT_DIM, H, W)
    return out


def run(x, coef, **spmd_kwargs):
    """Run on 8 NeuronCores; returns (output, BassKernelResults)."""
    nc = _get_module()
    in_maps = _prep_inputs(x, coef)
    res = run_bass_kernel_spmd(nc, in_maps, core_ids=list(range(8)), **spmd_kwargs)
    return _assemble(res.results), res


def kernel(x, coef):
    out, _ = run(x, coef)
    return out


# revision 4
# speedup vs baseline: 130.2780x; 1.1490x over previous
"""Trainium2 Bass kernel for nn_MemoryEfficientBSpline (linear B-spline / KAN layer).

Math: out[b,o,p] = sum_i sum_g Wt[b,i,p,g] * coef[b,o,i,g] where Wt is the
two-hot linear-interpolation weight of x[b,i,p] over a 6-knot grid on [-1,1].

Reformulation (hinge basis): with xc = clip(x,-1,1) and nx = 2.5*(xc+1) in [0,5],
the piecewise-linear spline f(nx) = sum_k beta_k * relu(nx - k) + c0 becomes

  out[b,o,p] = alpha[b,o] + sum_i W0[b,o,i]*xc[b,i,p]
             + sum_{k=1..4} sum_i Wk[b,o,i]*relu(xc[b,i,p] + 1 - 0.4k)

i.e. 5 dense [64x64] matmuls over cheap elementwise "hinge planes" of x.
Every device plane is clip(x, lo_k, 1) with the constant shift folded into the
output bias host-side, so all 5 planes are a single DVE tensor_scalar each.

Sharding: data-parallel over batch B=8, one batch per NeuronCore. Per core the
64x36864 pixel plane is folded to 128 partitions (two pixel-halves stacked), and
the 64x64 weights are block-diagonal-duplicated to 128x128 so one full-array
matmul handles both halves.

Dtype: fp16 end-to-end on device — x, hinge planes, weights, and y are all
fp16 (PSUM accumulates fp32; bias add + downcast happen on the ACT evict).
This halves HBM traffic vs fp32 (the old bottleneck) and lets the DVE run its
4x 2-byte perf mode, leaving the PE matmuls (1 cycle/column, same rate as
fp32r) as the bottleneck. fp16's 10 mantissa bits keep rel err ~1e-3.
"""
import numpy as np
from contextlib import ExitStack

import concourse.bass as bass
import concourse.tile as tile
from concourse import bacc, mybir
from concourse.bass_utils import run_bass_kernel_spmd

# Problem shapes (hardcoded per contract)
B, IN_DIM, H, W = 8, 64, 192, 192
OUT_DIM = 64
G = 6
P_TOT = H * W          # 36864 pixels
HALF = P_TOT // 2      # 18432 (folded columns)
NPART = 128
SLAB = 2048            # columns processed per pipeline iteration
N_SLABS = HALF // SLAB # 9
CHUNK = 512            # matmul moving-operand size (= one PSUM bank of fp32)
N_CHUNKS = SLAB // CHUNK
NK = 5                 # planes: clip(x, lo_k, 1) for lo = -1, -.6, -.2, .2, .6
LOS = (-1.0, -0.6, -0.2, 0.2, 0.6)

_f32 = mybir.dt.float32
_f16 = mybir.dt.float16
_Alu = mybir.AluOpType
_Act = mybir.ActivationFunctionType

_cached = None  # compiled Bass module, built once per process


def _build_module(n_reps=1):
    """n_reps>1 wraps the whole body in a hardware loop — used only for
    slope-based HW timing (dispatch noise >> exec time in this env)."""
    nc = bacc.Bacc("TRN2", target_bir_lowering=False, debug=False,
                   enable_asserts=False, num_devices=8)

    x_t = nc.dram_tensor("x", (NPART, HALF), _f16, kind="ExternalInput")
    w_t = nc.dram_tensor("wts", (NK, NPART, NPART), _f16, kind="ExternalInput")
    b_t = nc.dram_tensor("bias", (NPART, 1), _f32, kind="ExternalInput")
    y_t = nc.dram_tensor("y", (NPART, HALF), _f16, kind="ExternalOutput")

    with tile.TileContext(nc) as tc, ExitStack() as ctx:
        cpool = ctx.enter_context(tc.tile_pool(name="const", bufs=1))
        xpool = ctx.enter_context(tc.tile_pool(name="xin", bufs=3))
        ppool = ctx.enter_context(tc.tile_pool(name="planes", bufs=2))
        opool = ctx.enter_context(tc.tile_pool(name="oslab", bufs=3))
        psum = ctx.enter_context(tc.tile_pool(name="acc", bufs=2, space="PSUM"))

        # Constants: weights [128, 5*128] (block-diag per plane), bias
        wts = cpool.tile([NPART, NK * NPART], _f16)
        for k in range(NK):
            nc.sync.dma_start(wts[:, k*NPART:(k+1)*NPART], w_t[k])
        bias = cpool.tile([NPART, 1], _f32)
        nc.sync.dma_start(bias[:], b_t[:])

        def body():
            for s in range(N_SLABS):
                col0 = s * SLAB
                # Slab 0 is loaded and plane-computed in 512-col quarters so
                # the first matmul's inputs are ready sooner (pipeline fill).
                pieces = 4 if s == 0 else 1
                pw = SLAB // pieces
                xt = xpool.tile([NPART, SLAB], _f16)
                planes = [ppool.tile([NPART, SLAB], _f16, tag=f"p{k}",
                                     name=f"p{k}")
                          for k in range(NK)]
                for q in range(pieces):
                    sl = slice(q * pw, (q + 1) * pw)
                    nc.sync.dma_start(xt[:, sl], x_t[:, col0 + q*pw:col0 + (q+1)*pw])
                    for k in range(NK):
                        nc.vector.tensor_scalar(planes[k][:, sl], xt[:, sl],
                                                1.0, LOS[k], _Alu.min, _Alu.max)

                # Matmuls: 5 planes x 4 chunks accumulate into 4-bank psum
                acc = psum.tile([NPART, SLAB], _f32)
                for k in range(NK):
                    wk = wts[:, k*NPART:(k+1)*NPART]
                    for c in range(N_CHUNKS):
                        nc.tensor.matmul(acc[:, c*CHUNK:(c+1)*CHUNK], wk,
                                         planes[k][:, c*CHUNK:(c+1)*CHUNK],
                                         start=(k == 0), stop=(k == NK - 1))

                # Evict + bias on ACT, DMA out. The last slab evicts per psum
                # bank and DMAs in halves so the drain tail is one bank's
                # evict + half a slab's DMA instead of the whole slab's.
                ot = opool.tile([NPART, SLAB], _f16)
                if s == N_SLABS - 1:
                    for c in range(N_CHUNKS):
                        cs = slice(c*CHUNK, (c+1)*CHUNK)
                        nc.scalar.activation(ot[:, cs], acc[:, cs], _Act.Identity,
                                             bias=bias[:], scale=1.0)
                        if c % 2 == 1:
                            hs = slice((c-1)*CHUNK, (c+1)*CHUNK)
                            nc.sync.dma_start(y_t[:, col0+(c-1)*CHUNK:col0+(c+1)*CHUNK],
                                              ot[:, hs])
                else:
                    nc.scalar.activation(ot[:], acc[:], _Act.Identity,
                                         bias=bias[:], scale=1.0)
                    nc.sync.dma_start(y_t[:, col0:col0 + SLAB], ot[:])

        for _ in range(n_reps):
            body()

    nc.compile()
    return nc


def _get_module():
    global _cached
    if _cached is None:
        _cached = _build_module()
    return _cached


def _prep_inputs(x, coef):
    """Host-side shard + coefficient transform. Returns in_maps for 8 cores."""
    x = np.asarray(x, dtype=np.float32)
    c = np.asarray(coef, dtype=np.float64)            # [B, o, i, 6]
    d = np.diff(c, axis=-1)                           # [B, o, i, 5]
    beta = np.concatenate([d[..., :1], np.diff(d, axis=-1)], axis=-1)
    Wk = (2.5 * beta).astype(np.float16)              # [B, o, i, 5]
    Wk64 = Wk.astype(np.float64)
    # Device planes are clip(x, lo_k, 1) = relu(xc + c_k) - c_k with
    # c_k = 1 - 0.4k = -lo_k: fold the +c_k shift into the bias.
    alpha = (c[..., 0].sum(axis=2) + Wk64[..., 0].sum(axis=2)
             + sum((1.0 - 0.4 * k) * Wk64[..., k].sum(axis=2) for k in (1, 2, 3, 4))
             ).astype(np.float32)                     # [B, o]

    in_maps = []
    eye2 = np.eye(2, dtype=np.float16)
    for b in range(B):
        xb = x[b].reshape(IN_DIM, P_TOT)
        x_f = np.concatenate([xb[:, :HALF], xb[:, HALF:]], axis=0)  # [128, HALF]
        # lhsT[k][i, o] = Wk[b, o, i, k], block-diag duplicated to 128x128
        lhsT = np.einsum('oik->kio', Wk[b])           # [5, i, o]
        wts = np.kron(eye2, lhsT).astype(np.float16)  # [5, 128, 128]
        bias = np.tile(alpha[b], 2).reshape(NPART, 1).astype(np.float32)
        in_maps.append({
            "x": np.ascontiguousarray(x_f.astype(np.float16)),
            "wts": np.ascontiguousarray(wts),
            "bias": bias,
        })
    return in_maps


def _assemble(results):
    out = np.empty((B, OUT_DIM, H, W), dtype=np.float32)
    for b in range(B):
        y_f = results[b]["y"].astype(np.float32)       # [128, HALF]
        out[b] = np.concatenate([y_f[:OUT_DIM], y_f[OUT_DIM:]], axis=1).reshape(OUT_DIM, H, W)
    return out


def run(x, coef, **spmd_kwargs):
    """Run on 8 NeuronCores; returns (output, BassKernelResults)."""
    nc = _get_module()
    in_maps = _prep_inputs(x, coef)
    res = run_bass_kernel_spmd(nc, in_maps, core_ids=list(range(8)), **spmd_kwargs)
    return _assemble(res.results), res


def kernel(x, coef):
    out, _ = run(x, coef)
    return out
